# revision 39
# baseline (speedup 1.0000x reference)
"""NTM head addressing kernel for Trainium2 (8 NeuronCores, data-parallel over heads).

Shapes (hardcoded): B=4096 heads, N=2048 memory rows, C=128 memory cols.
Each core processes 512 heads as 4 tiles of 128 (partition dim = head).

Math restructuring vs the reference (exact up to fp rounding):
  - w = w_tilde^gamma / sum(w_tilde^gamma) is invariant to any per-head
    positive scale on w_tilde.  We therefore drop the softmax normalizer of
    s (divide taps by s1) and the (1-g) factor of the interpolation:
        u      = (g/(1-g)/sum_e) * e + w_prev          (e = exp(beta*sim))
        v      = (s0/s1)*u_{j-1} + u_j + (s2/s1)*u_{j+1}   (circular)
        w      = v^gamma / sum(v^gamma)
    with g/(1-g) = exp(g_raw) (sigmoid odds), s0/s1 = exp(s0_raw-s1_raw).
  - beta' = softplus(beta_raw)/||k|| is the per-partition scale of the ACT
    exp pass reading the matmul PSUM; sum_e comes free via accum_out.
    Likewise gamma' scales the final exp (accum_out=sum_y).
  - EPS terms are dropped: |denominators| >= ~2.5e-14 >> 1e-16 always.

Performance structure:
  - Host-side prep is limited to constants and O(B) scalar coefficients:
    M [N, C] is batch-replicated, so its row-normalization + transpose is
    weight prep (device loads MTh = (M/||M||_row)^T as fp16); the per-head
    softplus/exp/norm coefficients (5 scalars per head) ride a packed
    [128, 5*NT] table.  All O(B*N) work runs on device.
  - Everything on the DVE uses fp16 so tensor_scalar ops hit the 4x SIMD
    mode and tensor_tensor ops the 2x mode.  scalar_tensor_tensor has NO
    fast mode (measured dtype-blind 2.35us), so each former STT pass is a
    TS (4x) + TT (2x) pair: es = e*(rse*eg) fused two-scalar TS;
    u = es + wp; t1 = u_{-1}*s0'; c = t1 + u; t2 = u_{+1}*s2'; v = t2 + c.
    ln keeps an f32 result (exp amplifies errors by gamma); y is bf16
    (y = v^gamma spans ~1e-15..0.1 and underflows whole rows in fp16);
    e/es/pad/c/v/wout/wp are fp16 (ranges verified: e<=3.7, u<=0.02,
    v in [1e-4, 0.21] -- no fp16 over/underflow).
  - Output is written fp16 (halves the 4MB store) and upcast on host.
  - ACT (the only engine with Exp/Ln) and the DVE are co-critical at
    ~6.2-6.5us/tile; PSUM logits are double-buffered so PE matmuls for
    tile t+1 run under tile t's activations.  The input DMAs share the
    sync queue in priority order (kT+MTh halves packed into one transfer,
    w_prev tile-pairs behind), sm on the Pool queue; the last tile's out
    store issues are spread across sync/scalar/gpsimd queues to shorten
    the drain tail.
"""

import numpy as np

_B, _N, _C = 4096, 2048, 128
_NCORES = 8
_BS = _B // _NCORES      # 512 heads per core
_NT = _BS // 128         # 4 head tiles per core
_NQ = 4                  # PSUM-bank chunks per matmul tile

_built = None

_ONE_SET = "natural_log_exp_and_others"
_PINNED = {"Exp", "Ln", "Square", "Copy", "Identity"}


def _patch_act_tables():
    """Force Exp/Ln/Square/Copy onto the one table set that holds them all,
    so bacc's load inserter cannot thrash between per-function sets."""
    import concourse.bacc as bacc
    import concourse.hw_specs as hw_specs
    import concourse.mybir as mybir

    if getattr(bacc, "_ntm_table_patch", False):
        return
    orig = hw_specs.get_activation_tables
    pinned = {
        getattr(mybir.ActivationFunctionType, n)
        for n in _PINNED
        if hasattr(mybir.ActivationFunctionType, n)
    }

    def patched(module_arch):
        tables = orig(module_arch)
        out = {}
        for name, fns in tables.items():
            if name != _ONE_SET:
                fns = fns - pinned
            out[name] = fns
        return out

    bacc.get_activation_tables = patched
    bacc._ntm_table_patch = True


def _build():
    """Construct the (SPMD, per-core) Bass program."""
    import concourse.bass as bass
    import concourse.bacc as bacc
    import concourse.mybir as mybir
    import concourse.tile as tile

    _patch_act_tables()

    f32 = mybir.dt.float32
    f16 = mybir.dt.float16
    bf16 = mybir.dt.bfloat16
    AF = mybir.ActivationFunctionType
    OP = mybir.AluOpType

    nc = bacc.Bacc(
        "TRN2", target_bir_lowering=False, debug=False, num_devices=_NCORES
    )
    # kTA packs kT [C, BS] with the first half of MTh [C, N/2] so the
    # matmul inputs arrive in one transfer; wpP packs w_prev tile pairs.
    kTA_d = nc.declare_dram_parameter("kTA", [_C, _BS + _N // 2], f16, isOutput=False)
    MTB_d = nc.declare_dram_parameter("MTB", [_C, _N // 2], f16, isOutput=False)
    sm_d = nc.declare_dram_parameter("sm", [128, _NT * 5], f32, isOutput=False)
    wpP_d = [
        nc.declare_dram_parameter(f"wpP{i}", [128, 2 * _N], f16, isOutput=False)
        for i in range(2)
    ]
    out_d = nc.declare_dram_parameter("out", [_BS, _N], f16, isOutput=True)

    with tile.TileContext(nc) as tc:
        with (
            tc.tile_pool(name="const", bufs=1) as constp,
            tc.tile_pool(name="slab", bufs=2) as slabp,
            tc.tile_pool(name="mini", bufs=2) as minip,
            tc.tile_pool(name="psum", bufs=2, space=bass.MemorySpace.PSUM) as psump,
        ):
            # ---------------- input DMAs --------------------------------
            # Big loads share the sync queue: a single queue still fans out
            # over all 16 DMA engines, and queue order = priority (matmul
            # inputs first, w_prev pairs behind).  sm (the host-packed
            # per-head coefficients beta', eg, gamma', s0', s2') rides the
            # idle Pool queue; it is not needed until the first exp scale.
            sm = constp.tile([128, _NT * 5], f32)
            nc.gpsimd.dma_start(sm[:], sm_d[:])
            kTA = constp.tile([_C, _BS + _N // 2], f16)
            nc.sync.dma_start(kTA[:], kTA_d[:])
            MTB = constp.tile([_C, _N // 2], f16)
            nc.sync.dma_start(MTB[:], MTB_d[:])
            kT = kTA[:, 0:_BS]
            wpP = []
            for i in range(2):
                w = constp.tile([128, 2 * _N], f16, name=f"wpP{i}")
                nc.sync.dma_start(w[:], wpP_d[i][:])
                wpP.append(w)
            wps = [wpP[t // 2][:, (t % 2) * _N : (t % 2 + 1) * _N] for t in range(_NT)]
            # sm column blocks: [beta', eg, gamma', s0', s2'] (each _NT wide)
            def smc(block, t):
                return sm[:, block * _NT + t : block * _NT + t + 1]

            # warm the ACT function table during the DMA wait: the lazy
            # table load (1.3us) otherwise fires right before the first
            # real exp and can delay it
            dscr = minip.tile([128, 1], f32, tag="dscr", bufs=1)
            nc.scalar.activation(dscr[:], dscr[:], AF.Exp)

            # ---------------- main loop over 4 head tiles ----------------
            # For t<3 only the first wout half is scaled+stored inline; the
            # second halves are deferred past the whole loop, so the DVE
            # (priority = emission index, always picks the conv chain while
            # saturated) delivers every v earlier and runs the deferred
            # halves in its idle window under ln3/y3.  Half the out traffic
            # still streams early, keeping the final DMA drain short.
            deferred = []

            def emit_wout(t, y, sumy):
                r = minip.tile([128, 1], f32, tag="r", bufs=4)
                nc.vector.reciprocal(r[:], sumy[:])
                wout = slabp.tile([128, _N], f16, tag="wout", bufs=4)
                if t < _NT - 1:
                    nc.vector.tensor_scalar_mul(
                        wout[:, 0 : _N // 2], y[:, 0 : _N // 2], r[:]
                    )
                    nc.sync.dma_start(
                        out_d[:][t * 128 : (t + 1) * 128, 0 : _N // 2],
                        wout[:, 0 : _N // 2],
                    )
                    deferred.append((t, y, r, wout))
                else:
                    # last tile: quarters with issue cost spread across
                    # idle engine queues
                    H = _N // 4
                    issuers = [nc.sync, nc.scalar, nc.gpsimd, nc.sync]
                    for h in range(4):
                        sl = slice(h * H, (h + 1) * H)
                        nc.vector.tensor_scalar_mul(wout[:, sl], y[:, sl], r[:])
                        issuers[h].dma_start(
                            out_d[:][t * 128 : (t + 1) * 128, sl], wout[:, sl]
                        )

            for t in range(_NT):
                wp = wps[t]
                # cosine-sim logits (per PSUM bank) and e = exp(beta'*sim)
                logits = psump.tile([128, _N], f32, tag="ps")
                for q in range(_NQ):
                    if q < 2:
                        rhs = kTA[:, _BS + q * 512 : _BS + (q + 1) * 512]
                    else:
                        rhs = MTB[:, (q - 2) * 512 : (q - 1) * 512]
                    nc.tensor.matmul(
                        logits[:, q * 512 : (q + 1) * 512],
                        kT[:, t * 128 : (t + 1) * 128],
                        rhs,
                    )
                e = slabp.tile([128, _N], f16, tag="e", bufs=3)
                sume = minip.tile([128, 1], f32, tag="sume")
                nc.scalar.activation(
                    e[:], logits[:], AF.Exp,
                    scale=smc(0, t), accum_out=sume[:],
                )

                # es = (eg/sum_e)*e in ONE fused tensor_scalar (two scalar
                # ops: mult by 1/sum_e, mult by eg); u = es + wp.
                # NB tensor_scalar runs 4x and tensor_tensor 2x on packed
                # fp16, while scalar_tensor_tensor is dtype-blind (measured
                # 2.35us either way) -- hence TS+TT pairs, not STTs.
                rse = minip.tile([128, 1], f32, tag="rse")
                nc.vector.reciprocal(rse[:], sume[:])
                a = minip.tile([128, 1], f32, tag="a")
                nc.vector.tensor_mul(a[:], rse[:], smc(1, t))
                pad = slabp.tile([128, _N + 2], f16, tag="pad", bufs=3)
                # circular edge columns as 1-col STTs straight from e/wp:
                # ready together with es, so the u-add fires with no bubble
                # for the scheduler to steal another tile's work into
                nc.vector.scalar_tensor_tensor(
                    pad[:, 0:1], e[:, _N - 1 : _N], a[:],
                    wp[:, _N - 1 : _N], OP.mult, OP.add,
                )
                nc.vector.scalar_tensor_tensor(
                    pad[:, _N + 1 : _N + 2], e[:, 0:1], a[:],
                    wp[:, 0:1], OP.mult, OP.add,
                )
                es = slabp.tile([128, _N], f16, tag="es", bufs=3)
                nc.vector.tensor_scalar_mul(es[:], e[:], a[:])
                nc.vector.tensor_add(pad[:, 1 : _N + 1], es[:], wp[:])

                # circular 3-tap conv (middle tap normalized to 1):
                # v = s0'*u_{-1} + u + s2'*u_{+1}, as TS(4x)+TT(2x) pairs
                s0a = smc(3, t)
                s2a = smc(4, t)
                t1 = slabp.tile([128, _N], f16, tag="tmp", bufs=3)
                nc.vector.tensor_scalar_mul(t1[:], pad[:, 0:_N], s0a)
                c = slabp.tile([128, _N], f16, tag="c", bufs=3)
                nc.vector.tensor_add(c[:], t1[:], pad[:, 1 : _N + 1])
                t2 = slabp.tile([128, _N], f16, tag="tmp", bufs=3)
                nc.vector.tensor_scalar_mul(t2[:], pad[:, 2 : _N + 2], s2a)
                v = slabp.tile([128, _N], f16, tag="v", bufs=3)
                nc.vector.tensor_add(v[:], t2[:], c[:])

                # sharpen: y = v^gamma' = exp(gamma' * ln v), sum_y fused
                lw = slabp.tile([128, _N], f32, tag="lw")
                nc.scalar.activation(lw[:], v[:], AF.Ln)
                # y = v^gamma spans ~1e-15..0.08: needs bf16's exponent range
                # (fp16 flushes whole rows to 0); still 2-byte so the final
                # tensor_scalar keeps the 4x mode.
                y = slabp.tile([128, _N], bf16, tag="y", bufs=4)
                sumy = minip.tile([128, 1], f32, tag="sumy")
                nc.scalar.activation(
                    y[:], lw[:], AF.Exp,
                    scale=smc(2, t), accum_out=sumy[:],
                )
                emit_wout(t, y, sumy)

            # deferred second wout halves (tiles 0..2)
            issuers = [nc.scalar, nc.gpsimd, nc.sync]
            for i, (t, y, r, wout) in enumerate(deferred):
                hs = slice(_N // 2, _N)
                nc.vector.tensor_scalar_mul(wout[:, hs], y[:, hs], r[:])
                issuers[i].dma_start(
                    out_d[:][t * 128 : (t + 1) * 128, hs], wout[:, hs]
                )

    nc.compile()
    return nc


def _get_nc():
    global _built
    if _built is None:
        _built = _build()
    return _built


def _make_in_maps(k, beta, g, s, gamma, w_prev, M):
    # M is batch-replicated: row-normalize + transpose is host weight prep.
    Mf = np.asarray(M, dtype=np.float32)
    MT = (Mf / np.sqrt((Mf * Mf).sum(axis=1, keepdims=True))).T
    MTh = np.ascontiguousarray(MT.astype(np.float16))               # [128, N]
    # per-head coefficients (O(B) host scalar prep, like the kT transpose):
    # beta' = softplus(beta)/||k||, eg = sigmoid odds of g,
    # gamma' = 1 + softplus(gamma), taps s0/s1 = exp(s0-s1), s2/s1.
    kf = np.asarray(k, dtype=np.float32)
    knorm = np.sqrt((kf.astype(np.float16).astype(np.float32) ** 2).sum(1))
    bprime = np.logaddexp(0.0, beta[:, 0]) / knorm
    egv = np.exp(g[:, 0])
    gprime = 1.0 + np.logaddexp(0.0, gamma[:, 0])
    s0p = np.exp(s[:, 0] - s[:, 1])
    s2p = np.exp(s[:, 2] - s[:, 1])
    MTA, MTB = MTh[:, : _N // 2], np.ascontiguousarray(MTh[:, _N // 2 :])
    in_maps = []
    for c in range(_NCORES):
        sl = slice(c * _BS, (c + 1) * _BS)
        ks = kf[sl]
        kTs = ks.T.astype(np.float16)                               # [128,512]
        # packed per-head coefficients: [128, 5*NT]; col block order:
        # beta', eg, gamma', s0', s2' (each NT wide; head = t*128 + p)
        def cols(x):
            return np.ascontiguousarray(x.reshape(_NT, 128).T, dtype=np.float32)
        sm = np.concatenate(
            [
                cols(bprime[sl]),
                cols(egv[sl]),
                cols(gprime[sl]),
                cols(s0p[sl]),
                cols(s2p[sl]),
            ],
            axis=1,
        )
        wpc = w_prev[sl].astype(np.float16)                         # [512, N]
        in_maps.append(
            {
                "kTA": np.ascontiguousarray(np.concatenate([kTs, MTA], axis=1)),
                "MTB": MTB,
                "sm": np.ascontiguousarray(sm),
                "wpP0": np.ascontiguousarray(
                    np.concatenate([wpc[0:128], wpc[128:256]], axis=1)
                ),
                "wpP1": np.ascontiguousarray(
                    np.concatenate([wpc[256:384], wpc[384:512]], axis=1)
                ),
            }
        )
    return in_maps


def kernel(k, beta, g, s, gamma, w_prev, M, _trace=False, _tmpdir=None):
    from concourse.bass_utils import run_bass_kernel_spmd

    nc = _get_nc()
    in_maps = _make_in_maps(
        np.asarray(k), np.asarray(beta), np.asarray(g), np.asarray(s),
        np.asarray(gamma), np.asarray(w_prev), np.asarray(M),
    )
    res = run_bass_kernel_spmd(
        nc, in_maps, list(range(_NCORES)), trace=_trace, tmpdir=_tmpdir
    )
    out = np.concatenate(
        [res.results[c]["out"].astype(np.float32) for c in range(_NCORES)], axis=0
    )
    if _trace:
        kernel._last_results = res
    return out


# revision 40
# speedup vs baseline: 1.0022x; 1.0022x over previous
"""NTM head addressing kernel for Trainium2 (8 NeuronCores, data-parallel over heads).

Shapes (hardcoded): B=4096 heads, N=2048 memory rows, C=128 memory cols.
Each core processes 512 heads as 4 tiles of 128 (partition dim = head).

Math restructuring vs the reference (exact up to fp rounding):
  - w = w_tilde^gamma / sum(w_tilde^gamma) is invariant to any per-head
    positive scale on w_tilde.  We therefore drop the softmax normalizer of
    s (divide taps by s1) and the (1-g) factor of the interpolation:
        u      = (g/(1-g)/sum_e) * e + w_prev          (e = exp(beta*sim))
        v      = (s0/s1)*u_{j-1} + u_j + (s2/s1)*u_{j+1}   (circular)
        w      = v^gamma / sum(v^gamma)
    with g/(1-g) = exp(g_raw) (sigmoid odds), s0/s1 = exp(s0_raw-s1_raw).
  - beta' = softplus(beta_raw)/||k|| is the per-partition scale of the ACT
    exp pass reading the matmul PSUM; sum_e comes free via accum_out.
    Likewise gamma' scales the final exp (accum_out=sum_y).
  - EPS terms are dropped: |denominators| >= ~2.5e-14 >> 1e-16 always.

Performance structure:
  - Host-side prep is limited to constants and O(B) scalar coefficients:
    M [N, C] is batch-replicated, so its row-normalization + transpose is
    weight prep (device loads MTh = (M/||M||_row)^T as fp16); the per-head
    softplus/exp/norm coefficients (5 scalars per head) ride a packed
    [128, 5*NT] table.  All O(B*N) work runs on device.
  - Everything on the DVE uses fp16 so tensor_scalar ops hit the 4x SIMD
    mode and tensor_tensor ops the 2x mode.  scalar_tensor_tensor has NO
    fast mode (measured dtype-blind 2.35us), so each former STT pass is a
    TS (4x) + TT (2x) pair: es = e*(rse*eg) fused two-scalar TS;
    u = es + wp; t1 = u_{-1}*s0'; c = t1 + u; t2 = u_{+1}*s2'; v = t2 + c.
    ln keeps an f32 result (exp amplifies errors by gamma); y is bf16
    (y = v^gamma spans ~1e-15..0.1 and underflows whole rows in fp16);
    e/es/pad/c/v/wout/wp are fp16 (ranges verified: e<=3.7, u<=0.02,
    v in [1e-4, 0.21] -- no fp16 over/underflow).
  - Output is written fp16 (halves the 4MB store) and upcast on host.
  - ACT (the only engine with Exp/Ln) and the DVE are co-critical at
    ~6.2-6.5us/tile; PSUM logits are double-buffered so PE matmuls for
    tile t+1 run under tile t's activations.  The input DMAs share the
    sync queue in priority order (kT+MTh halves packed into one transfer,
    w_prev tile-pairs behind), sm on the Pool queue; the last tile's out
    store issues are spread across sync/scalar/gpsimd queues to shorten
    the drain tail.
"""

import numpy as np

_B, _N, _C = 4096, 2048, 128
_NCORES = 8
_BS = _B // _NCORES      # 512 heads per core
_NT = _BS // 128         # 4 head tiles per core
_NQ = 4                  # PSUM-bank chunks per matmul tile

_built = None

_ONE_SET = "natural_log_exp_and_others"
_PINNED = {"Exp", "Ln", "Square", "Copy", "Identity"}


def _patch_act_tables():
    """Force Exp/Ln/Square/Copy onto the one table set that holds them all,
    so bacc's load inserter cannot thrash between per-function sets."""
    import concourse.bacc as bacc
    import concourse.hw_specs as hw_specs
    import concourse.mybir as mybir

    if getattr(bacc, "_ntm_table_patch", False):
        return
    orig = hw_specs.get_activation_tables
    pinned = {
        getattr(mybir.ActivationFunctionType, n)
        for n in _PINNED
        if hasattr(mybir.ActivationFunctionType, n)
    }

    def patched(module_arch):
        tables = orig(module_arch)
        out = {}
        for name, fns in tables.items():
            if name != _ONE_SET:
                fns = fns - pinned
            out[name] = fns
        return out

    bacc.get_activation_tables = patched
    bacc._ntm_table_patch = True


def _build():
    """Construct the (SPMD, per-core) Bass program."""
    import concourse.bass as bass
    import concourse.bacc as bacc
    import concourse.mybir as mybir
    import concourse.tile as tile

    _patch_act_tables()

    f32 = mybir.dt.float32
    f16 = mybir.dt.float16
    bf16 = mybir.dt.bfloat16
    AF = mybir.ActivationFunctionType
    OP = mybir.AluOpType

    nc = bacc.Bacc(
        "TRN2", target_bir_lowering=False, debug=False, num_devices=_NCORES
    )
    # kTA packs kT [C, BS] with the first half of MTh [C, N/2] so the
    # matmul inputs arrive in one transfer; wpP packs w_prev tile pairs.
    kTA_d = nc.declare_dram_parameter("kTA", [_C, _BS + _N // 2], f16, isOutput=False)
    MTB_d = nc.declare_dram_parameter("MTB", [_C, _N // 2], f16, isOutput=False)
    sm_d = nc.declare_dram_parameter("sm", [128, _NT * 5], f32, isOutput=False)
    wpP_d = [
        nc.declare_dram_parameter(f"wpP{i}", [128, 2 * _N], f16, isOutput=False)
        for i in range(2)
    ]
    out_d = nc.declare_dram_parameter("out", [_BS, _N], f16, isOutput=True)

    with tile.TileContext(nc) as tc:
        with (
            tc.tile_pool(name="const", bufs=1) as constp,
            tc.tile_pool(name="slab", bufs=2) as slabp,
            tc.tile_pool(name="mini", bufs=2) as minip,
            tc.tile_pool(name="psum", bufs=2, space=bass.MemorySpace.PSUM) as psump,
        ):
            # ---------------- input DMAs --------------------------------
            # Big loads share the sync queue: a single queue still fans out
            # over all 16 DMA engines, and queue order = priority (matmul
            # inputs first, w_prev pairs behind).  sm (the host-packed
            # per-head coefficients beta', eg, gamma', s0', s2') rides the
            # idle Pool queue; it is not needed until the first exp scale.
            sm = constp.tile([128, _NT * 5], f32)
            nc.gpsimd.dma_start(sm[:], sm_d[:])
            kTA = constp.tile([_C, _BS + _N // 2], f16)
            nc.sync.dma_start(kTA[:], kTA_d[:])
            MTB = constp.tile([_C, _N // 2], f16)
            nc.sync.dma_start(MTB[:], MTB_d[:])
            kT = kTA[:, 0:_BS]
            wpP = []
            for i in range(2):
                w = constp.tile([128, 2 * _N], f16, name=f"wpP{i}")
                nc.sync.dma_start(w[:], wpP_d[i][:])
                wpP.append(w)
            wps = [wpP[t // 2][:, (t % 2) * _N : (t % 2 + 1) * _N] for t in range(_NT)]
            # sm column blocks: [beta', eg, gamma', s0', s2'] (each _NT wide)
            def smc(block, t):
                return sm[:, block * _NT + t : block * _NT + t + 1]

            # warm the ACT function table during the DMA wait: the lazy
            # table load (1.3us) otherwise fires right before the first
            # real exp and can delay it
            dscr = minip.tile([128, 1], f32, tag="dscr", bufs=1)
            nc.scalar.activation(dscr[:], dscr[:], AF.Exp)

            # ---------------- main loop over 4 head tiles ----------------
            # For t<3 only the first wout half is scaled+stored inline; the
            # second halves are deferred past the whole loop, so the DVE
            # (priority = emission index, always picks the conv chain while
            # saturated) delivers every v earlier and runs the deferred
            # halves in its idle window under ln3/y3.  Half the out traffic
            # still streams early, keeping the final DMA drain short.
            deferred = []

            def emit_wout(t, y, sumy):
                r = minip.tile([128, 1], f32, tag="r", bufs=4)
                nc.vector.reciprocal(r[:], sumy[:])
                wout = slabp.tile([128, _N], f16, tag="wout", bufs=4)
                if t < _NT - 1:
                    # inline 3/4 of the row (streams out early, keeps the
                    # end-of-kernel DMA drain short); defer only the last
                    # quarter past the loop
                    Q = 3 * _N // 4
                    nc.vector.tensor_scalar_mul(wout[:, 0:Q], y[:, 0:Q], r[:])
                    nc.sync.dma_start(
                        out_d[:][t * 128 : (t + 1) * 128, 0:Q], wout[:, 0:Q]
                    )
                    deferred.append((t, y, r, wout))
                else:
                    # last tile: quarters with issue cost spread across
                    # idle engine queues
                    H = _N // 4
                    issuers = [nc.sync, nc.scalar, nc.gpsimd, nc.sync]
                    for h in range(4):
                        sl = slice(h * H, (h + 1) * H)
                        nc.vector.tensor_scalar_mul(wout[:, sl], y[:, sl], r[:])
                        issuers[h].dma_start(
                            out_d[:][t * 128 : (t + 1) * 128, sl], wout[:, sl]
                        )

            for t in range(_NT):
                wp = wps[t]
                # cosine-sim logits (per PSUM bank) and e = exp(beta'*sim)
                logits = psump.tile([128, _N], f32, tag="ps")
                for q in range(_NQ):
                    if q < 2:
                        rhs = kTA[:, _BS + q * 512 : _BS + (q + 1) * 512]
                    else:
                        rhs = MTB[:, (q - 2) * 512 : (q - 1) * 512]
                    nc.tensor.matmul(
                        logits[:, q * 512 : (q + 1) * 512],
                        kT[:, t * 128 : (t + 1) * 128],
                        rhs,
                    )
                e = slabp.tile([128, _N], f16, tag="e", bufs=3)
                sume = minip.tile([128, 1], f32, tag="sume")
                nc.scalar.activation(
                    e[:], logits[:], AF.Exp,
                    scale=smc(0, t), accum_out=sume[:],
                )

                # es = (eg/sum_e)*e in ONE fused tensor_scalar (two scalar
                # ops: mult by 1/sum_e, mult by eg); u = es + wp.
                # NB tensor_scalar runs 4x and tensor_tensor 2x on packed
                # fp16, while scalar_tensor_tensor is dtype-blind (measured
                # 2.35us either way) -- hence TS+TT pairs, not STTs.
                rse = minip.tile([128, 1], f32, tag="rse")
                nc.vector.reciprocal(rse[:], sume[:])
                a = minip.tile([128, 1], f32, tag="a")
                nc.vector.tensor_mul(a[:], rse[:], smc(1, t))
                pad = slabp.tile([128, _N + 2], f16, tag="pad", bufs=3)
                # circular edge columns as 1-col STTs straight from e/wp:
                # ready together with es, so the u-add fires with no bubble
                # for the scheduler to steal another tile's work into
                nc.vector.scalar_tensor_tensor(
                    pad[:, 0:1], e[:, _N - 1 : _N], a[:],
                    wp[:, _N - 1 : _N], OP.mult, OP.add,
                )
                nc.vector.scalar_tensor_tensor(
                    pad[:, _N + 1 : _N + 2], e[:, 0:1], a[:],
                    wp[:, 0:1], OP.mult, OP.add,
                )
                es = slabp.tile([128, _N], f16, tag="es", bufs=3)
                nc.vector.tensor_scalar_mul(es[:], e[:], a[:])
                nc.vector.tensor_add(pad[:, 1 : _N + 1], es[:], wp[:])

                # circular 3-tap conv (middle tap normalized to 1):
                # v = s0'*u_{-1} + u + s2'*u_{+1}, as TS(4x)+TT(2x) pairs
                s0a = smc(3, t)
                s2a = smc(4, t)
                t1 = slabp.tile([128, _N], f16, tag="tmp", bufs=3)
                nc.vector.tensor_scalar_mul(t1[:], pad[:, 0:_N], s0a)
                c = slabp.tile([128, _N], f16, tag="c", bufs=3)
                nc.vector.tensor_add(c[:], t1[:], pad[:, 1 : _N + 1])
                t2 = slabp.tile([128, _N], f16, tag="tmp", bufs=3)
                nc.vector.tensor_scalar_mul(t2[:], pad[:, 2 : _N + 2], s2a)
                v = slabp.tile([128, _N], f16, tag="v", bufs=3)
                nc.vector.tensor_add(v[:], t2[:], c[:])

                # sharpen: y = v^gamma' = exp(gamma' * ln v), sum_y fused
                lw = slabp.tile([128, _N], f32, tag="lw")
                nc.scalar.activation(lw[:], v[:], AF.Ln)
                # y = v^gamma spans ~1e-15..0.08: needs bf16's exponent range
                # (fp16 flushes whole rows to 0); still 2-byte so the final
                # tensor_scalar keeps the 4x mode.
                y = slabp.tile([128, _N], bf16, tag="y", bufs=4)
                sumy = minip.tile([128, 1], f32, tag="sumy")
                nc.scalar.activation(
                    y[:], lw[:], AF.Exp,
                    scale=smc(2, t), accum_out=sumy[:],
                )
                emit_wout(t, y, sumy)

            # deferred second wout halves (tiles 0..2)
            issuers = [nc.scalar, nc.gpsimd, nc.sync]
            for i, (t, y, r, wout) in enumerate(deferred):
                hs = slice(3 * _N // 4, _N)
                nc.vector.tensor_scalar_mul(wout[:, hs], y[:, hs], r[:])
                issuers[i].dma_start(
                    out_d[:][t * 128 : (t + 1) * 128, hs], wout[:, hs]
                )

    nc.compile()
    return nc


def _get_nc():
    global _built
    if _built is None:
        _built = _build()
    return _built


def _make_in_maps(k, beta, g, s, gamma, w_prev, M):
    # M is batch-replicated: row-normalize + transpose is host weight prep.
    Mf = np.asarray(M, dtype=np.float32)
    MT = (Mf / np.sqrt((Mf * Mf).sum(axis=1, keepdims=True))).T
    MTh = np.ascontiguousarray(MT.astype(np.float16))               # [128, N]
    # per-head coefficients (O(B) host scalar prep, like the kT transpose):
    # beta' = softplus(beta)/||k||, eg = sigmoid odds of g,
    # gamma' = 1 + softplus(gamma), taps s0/s1 = exp(s0-s1), s2/s1.
    kf = np.asarray(k, dtype=np.float32)
    knorm = np.sqrt((kf.astype(np.float16).astype(np.float32) ** 2).sum(1))
    bprime = np.logaddexp(0.0, beta[:, 0]) / knorm
    egv = np.exp(g[:, 0])
    gprime = 1.0 + np.logaddexp(0.0, gamma[:, 0])
    s0p = np.exp(s[:, 0] - s[:, 1])
    s2p = np.exp(s[:, 2] - s[:, 1])
    MTA, MTB = MTh[:, : _N // 2], np.ascontiguousarray(MTh[:, _N // 2 :])
    in_maps = []
    for c in range(_NCORES):
        sl = slice(c * _BS, (c + 1) * _BS)
        ks = kf[sl]
        kTs = ks.T.astype(np.float16)                               # [128,512]
        # packed per-head coefficients: [128, 5*NT]; col block order:
        # beta', eg, gamma', s0', s2' (each NT wide; head = t*128 + p)
        def cols(x):
            return np.ascontiguousarray(x.reshape(_NT, 128).T, dtype=np.float32)
        sm = np.concatenate(
            [
                cols(bprime[sl]),
                cols(egv[sl]),
                cols(gprime[sl]),
                cols(s0p[sl]),
                cols(s2p[sl]),
            ],
            axis=1,
        )
        wpc = w_prev[sl].astype(np.float16)                         # [512, N]
        in_maps.append(
            {
                "kTA": np.ascontiguousarray(np.concatenate([kTs, MTA], axis=1)),
                "MTB": MTB,
                "sm": np.ascontiguousarray(sm),
                "wpP0": np.ascontiguousarray(
                    np.concatenate([wpc[0:128], wpc[128:256]], axis=1)
                ),
                "wpP1": np.ascontiguousarray(
                    np.concatenate([wpc[256:384], wpc[384:512]], axis=1)
                ),
            }
        )
    return in_maps


def kernel(k, beta, g, s, gamma, w_prev, M, _trace=False, _tmpdir=None):
    from concourse.bass_utils import run_bass_kernel_spmd

    nc = _get_nc()
    in_maps = _make_in_maps(
        np.asarray(k), np.asarray(beta), np.asarray(g), np.asarray(s),
        np.asarray(gamma), np.asarray(w_prev), np.asarray(M),
    )
    res = run_bass_kernel_spmd(
        nc, in_maps, list(range(_NCORES)), trace=_trace, tmpdir=_tmpdir
    )
    out = np.concatenate(
        [res.results[c]["out"].astype(np.float32) for c in range(_NCORES)], axis=0
    )
    if _trace:
        kernel._last_results = res
    return out
